# revision 24
# baseline (speedup 1.0000x reference)
"""Trainium2 Bass kernel for MHA with query-axis softmax (nn_MHA_2568390443327).

Reference computation (B=4, N=2048, DIM=1024, 16 heads x 64):
    qkv = x @ w_qkv ; q,k,v = split(qkv)
    scores = (q @ k^T) * scale            # [b,h,i(query),j(key)]
    attn = softmax(scores, axis=QUERY)    # normalized over i, per key j
    y = attn @ v ; out = y @ w_out + b_out

Sharding (8 cores): batch (4) x head-half (2). Each core: its batch's x
(pre-transposed), qkv weight columns + w_out rows for its 8 heads, producing
a partial [OUT, N] f16 output. Host sums the two head-half partials per batch,
adds b_out, transposes back.

Per-core schedule: act (exp) engine is the bottleneck (~284us of [128,1024]
Exp tiles); everything else is laced around its cadence:
  - scores computed transposed S_T[j,i] so the query-axis softmax denominator
    is a free-axis reduce; exp tiles go to SBUF f16, denominators via DVE
    tensor_reduce (pass A) + reduce/add/recip (pass B), v-rescale on GpSimd.
  - attention runs per head-pair in two i-half passes (A: i 0..1023,
    B: 1024..2047); pass-A exp tiles are retained in SBUF so attn@v for both
    halves trails pass B by 2 slots, accumulating y in a 4-bank PSUM region.
  - the y PSUM region is idle during pass A; all projection matmuls (v, next
    pair's q/k, final out-proj) rotate through that same 4-bank pool, paced
    ~2 groups per act slot to keep the PE dense (HAM clock-gate stays 8/8).
  - scores ring: 2 x [128,1024] PSUM tiles (the other 4 banks).
All matmul operands f16 (full PE rate, fp32 PSUM accumulation).
"""

import os
import numpy as np

# ---------------------------------------------------------------------------
B = 4
N = 2048          # sequence length
F = 1024          # model dim (contraction for qkv proj)
DH = 64           # head dim
HH = 8            # heads per core
CH = HH * DH      # 512: per-core hidden
OUT = 1024        # output dim
SCALE = 0.125     # 1/sqrt(64)
N_CORES = 8

P = 128           # partitions
PAIRS = 4         # head pairs per core
NT = N // P       # 16 j-tiles
KT = F // P       # 8 k-tiles
HW = N // 2       # 1024: i-half width


def _build_nc():
    import concourse.bass as bass  # noqa: F401
    import concourse.mybir as mybir
    from concourse import bacc
    from concourse.tile import TileContext

    f32 = mybir.dt.float32
    f16 = mybir.dt.float16
    EXP = mybir.ActivationFunctionType.Exp
    AX = mybir.AxisListType.X
    ADD = mybir.AluOpType.add

    nc = bacc.Bacc(None, target_bir_lowering=False)

    xT = nc.declare_dram_parameter("xT", [F, N], f16, isOutput=False)
    wqkv = nc.declare_dram_parameter("wqkv", [F, 3 * CH], f16, isOutput=False)
    wout = nc.declare_dram_parameter("wout", [CH, OUT], f16, isOutput=False)
    outT = nc.declare_dram_parameter("outT", [OUT, N], f16, isOutput=True)

    debug = bool(os.environ.get("MHA_DEBUG"))
    dbg = {}
    if debug:
        for nm, shp, dt in [("d_q0", [P, N], f16), ("d_k0", [P, N], f16),
                            ("d_v", [P, NT * CH], f16), ("d_e00", [P, HW], f16),
                            ("d_e00B", [P, HW], f16), ("d_dA0", [P, NT], f16),
                            ("d_vp00", [P, DH], f16), ("d_y0", [P, N], f16)]:
            dbg[nm] = nc.declare_dram_parameter(nm, shp, dt, isOutput=True)

    with TileContext(nc) as tc:
        with (
            tc.tile_pool(name="p_x", bufs=1) as p_x,
            tc.tile_pool(name="p_w", bufs=1) as p_w,
            tc.tile_pool(name="p_wout", bufs=1) as p_wout,
            tc.tile_pool(name="p_qk", bufs=2) as p_qk,
            tc.tile_pool(name="p_v", bufs=1) as p_v,
            tc.tile_pool(name="p_eA", bufs=1) as p_eA,
            tc.tile_pool(name="p_eB", bufs=6) as p_eB,
            tc.tile_pool(name="p_ysb", bufs=1) as p_ysb,
            tc.tile_pool(name="p_dA", bufs=2) as p_dA,
            tc.tile_pool(name="p_sm", bufs=6) as p_sm,
            tc.tile_pool(name="p_vp", bufs=6) as p_vp,
            tc.tile_pool(name="p_osb", bufs=2) as p_osb,
            tc.tile_pool(name="psS", bufs=2, space="PSUM") as psS,
            tc.tile_pool(name="psY", bufs=2, space="PSUM") as psY,
        ):
            xt = [p_x.tile([P, N], f16, tag=f"x{k}", name=f"x{k}")
                  for k in range(KT)]
            wt = [p_w.tile([P, 3 * CH], f16, tag=f"w{k}", name=f"w{k}")
                  for k in range(KT)]
            wout_sb = [p_wout.tile([P, OUT], f16, tag=f"wo{c}", name=f"wo{c}")
                       for c in range(CH // P)]
            vnat = p_v.tile([P, NT * CH], f16, tag="v", name="vnat")
            y_sb = [p_ysb.tile([P, N], f16, tag=f"y{p_}", name=f"y{p_}")
                    for p_ in range(PAIRS)]

            for k in range(KT):
                nc.sync.dma_start(out=xt[k], in_=xT[k * P:(k + 1) * P, :])
                nc.sync.dma_start(out=wt[k], in_=wqkv[k * P:(k + 1) * P, :])
            for c in range(CH // P):
                nc.sync.dma_start(out=wout_sb[c],
                                  in_=wout[c * P:(c + 1) * P, :])

            # ---------------- projection helpers (psY rotations) ----------
            qk_t = {}    # pair -> (qT tile, kT tile)

            def qk_rot_units(pr):
                """q/k projection for a pair as psY [128,1024] rotations.
                Order (q-h0, k-h0, q-h1, k-h1): pass-A scores need only the
                h0 halves of q plus k, so attention starts 2 rotations in."""
                dsts = {}
                for sec in (0, 1):
                    dsts[sec] = p_qk.tile([P, N], f16,
                                          tag="q" if sec == 0 else "k",
                                          name=f"{'qk'[sec]}T{pr}")
                    qk_t.setdefault(pr, {})[sec] = dsts[sec]
                for h in range(2):
                    for sec in (0, 1):
                        dst = dsts[sec]
                        st = {}

                        def alloc(sec=sec, h=h):
                            st['ps'] = psY.tile([P, HW], f32, tag="Y",
                                                name=f"qk{pr}_{sec}_{h}")

                        def grp(nch, sec=sec):
                            ps = st['ps']
                            for k in range(KT):
                                nc.tensor.matmul(
                                    ps[:, (nch % 2) * 512:(nch % 2 + 1) * 512],
                                    lhsT=wt[k][:, sec * CH + pr * P:
                                               sec * CH + (pr + 1) * P],
                                    rhs=xt[k][:, nch * 512:(nch + 1) * 512],
                                    start=(k == 0), stop=(k == KT - 1))

                        def cast(dst=dst, h=h):
                            nc.vector.tensor_copy(
                                dst[:, h * HW:(h + 1) * HW], st['ps'])

                        yield ('alloc', alloc)
                        for nch in (2 * h, 2 * h + 1):
                            yield ('group', lambda nch=nch, g=grp: g(nch))
                        yield ('cast', cast)

            def v_rot_units():
                for vbase in range(0, NT, 2):
                    st = {}

                    def alloc(vbase=vbase):
                        st['ps'] = psY.tile([P, HW], f32, tag="Y",
                                            name=f"v{vbase}")

                    def grp(q, vbase=vbase):
                        j = vbase + q
                        ps = st['ps']
                        for k in range(KT):
                            nc.tensor.matmul(
                                ps[:, q * 512:(q + 1) * 512],
                                lhsT=xt[k][:, j * P:(j + 1) * P],
                                rhs=wt[k][:, 2 * CH:3 * CH],
                                start=(k == 0), stop=(k == KT - 1))

                    def cast(vbase=vbase):
                        nc.vector.tensor_copy(
                            vnat[:, vbase * CH:(vbase + 2) * CH], st['ps'])

                    yield ('alloc', alloc)
                    for q in range(2):
                        yield ('group', lambda q=q, g=grp: g(q))
                    yield ('cast', cast)

            def run_units(units):
                """Emit all units of a projection immediately."""
                for kind, fn in units:
                    fn()

            class Pacer:
                """Paced emission of projection units into attention slots."""
                def __init__(self):
                    self.units = []
                    self.i = 0

                def extend(self, gen):
                    self.units.extend(gen)

                def step(self, ngroups):
                    """Emit until `ngroups` matmul groups are emitted."""
                    g = 0
                    while self.i < len(self.units) and g < ngroups:
                        kind, fn = self.units[self.i]
                        fn()
                        self.i += 1
                        if kind == 'group':
                            g += 1

                def drain(self):
                    while self.i < len(self.units):
                        self.units[self.i][1]()
                        self.i += 1

            # ---------------- attention ----------------------------------
            # lead-in: only q-h0 + k-h0 of pair 0 (8 units); the h1
            # rotations go to the front of pair 0's pacer queue
            units0 = list(qk_rot_units(0))
            run_units(units0[:8])

            eA = {}      # (j, ho) -> pass-A exp tile (per-pair reuse)
            state = {}   # per (pr): denA tiles, rec/vp handles

            def emit_scores(pr, j, half, ho):
                sps = psS.tile([P, HW], f32, tag="S",
                               name=f"s{pr}_{j}_{half}_{ho}")
                qt = qk_t[pr][0]
                kt = qk_t[pr][1]
                for c2 in range(2):
                    nc.tensor.matmul(
                        sps[:, c2 * 512:(c2 + 1) * 512],
                        lhsT=kt[ho:ho + DH, j * P:(j + 1) * P],
                        rhs=qt[ho:ho + DH,
                               half * HW + c2 * 512:half * HW + (c2 + 1) * 512],
                        start=True, stop=True, tile_position=(ho, 0))
                return sps

            def emit_act(pr, j, half, ho, sps):
                if half == 0:
                    et = p_eA.tile([P, HW], f16, tag=f"eA{j}_{ho}",
                                   name=f"eA{j}_{ho}")
                    eA[(j, ho)] = et
                else:
                    et = p_eB.tile([P, HW], f16, tag="eB", name="eB")
                nc.scalar.activation(et, sps, EXP, scale=SCALE)
                return et

            def emit_av(pr, j, ho, y_ps, eBt, vpt):
                for hf, et in ((0, eA[(j, ho)]), (1, eBt)):
                    for c2 in range(2):
                        nc.tensor.matmul(
                            y_ps[hf][ho:ho + DH, c2 * 512:(c2 + 1) * 512],
                            lhsT=vpt,
                            rhs=et[:, c2 * 512:(c2 + 1) * 512],
                            start=(j == 0), stop=(j == NT - 1),
                            tile_position=(0, ho), skip_group_check=True)

            for pr in range(PAIRS):
                pacer = Pacer()
                if pr == 0:
                    pacer.extend(units0[8:])
                    pacer.extend(v_rot_units())
                    pacer.extend(qk_rot_units(1))
                elif pr < PAIRS - 1:
                    pacer.extend(qk_rot_units(pr + 1))

                dA = {ho: p_dA.tile([P, NT], f16, tag=f"dA{ho}",
                                    name=f"dA{pr}_{ho}")
                      for ho in (0, DH)}
                prev = state.get(pr - 1)

                # ---- pass A: i in [0, 1024) : scores + exp + denA
                for j in range(NT):
                    for ho in (0, DH):
                        sps = emit_scores(pr, j, 0, ho)
                        et = emit_act(pr, j, 0, ho, sps)
                        with nc.allow_low_precision("f16 den keeps DVE 2x"):
                            nc.vector.tensor_reduce(
                                dA[ho][:, j:j + 1], et, AX, ADD)
                    # trailing work of the previous pair goes right after
                    # j=0's scores so the act engine never waits at the
                    # pair boundary
                    if prev is not None and j == 0:
                        for pj in (NT - 2, NT - 1):
                            for ho in (0, DH):
                                emit_av(pr - 1, pj, ho, prev['y_ps'],
                                        prev['eB'][(pj, ho)],
                                        prev['vp'][(pj, ho)])
                        for hf in range(2):
                            nc.vector.tensor_copy(
                                y_sb[pr - 1][:, hf * HW:(hf + 1) * HW],
                                prev['y_ps'][hf])
                        if debug and pr == 1:
                            nc.sync.dma_start(out=dbg["d_y0"][:, :],
                                              in_=y_sb[0])
                        state.pop(pr - 1)
                    if debug and pr == 0 and j == 0:
                        nc.sync.dma_start(out=dbg["d_e00"][:, :], in_=eA[(0, 0)])
                    pacer.step(2 if pr == 0 else 1)

                # ---- pass B: i in [1024, 2048) : + den total + av(j-2)
                if debug and pr == 0:
                    nc.sync.dma_start(out=dbg["d_q0"][:, :], in_=qk_t[0][0])
                    nc.sync.dma_start(out=dbg["d_k0"][:, :], in_=qk_t[0][1])
                    nc.sync.dma_start(out=dbg["d_v"][:, :], in_=vnat)
                    nc.sync.dma_start(out=dbg["d_dA0"][:, :], in_=dA[0])
                cur = {'eB': {}, 'vp': {},
                       'y_ps': [psY.tile([P, HW], f32, tag="Y",
                                         name=f"yps{pr}_{hf}")
                                for hf in range(2)]}
                state[pr] = cur
                for j in range(NT):
                    for ho in (0, DH):
                        sps = emit_scores(pr, j, 1, ho)
                        et = emit_act(pr, j, 1, ho, sps)
                        cur['eB'][(j, ho)] = et
                        dB = p_sm.tile([P, 1], f16, tag="dB", name="dB")
                        with nc.allow_low_precision("f16 den keeps DVE 2x"):
                            nc.vector.tensor_reduce(dB, et, AX, ADD)
                        dtot = p_sm.tile([P, 1], f32, tag="dt", name="dt")
                        nc.vector.tensor_add(dtot, dA[ho][:, j:j + 1], dB)
                        rec = p_sm.tile([P, 1], f32, tag="rc", name="rc")
                        nc.vector.reciprocal(rec, dtot)
                        vpt = p_vp.tile([P, DH], f16, tag="vp", name="vp")
                        c0 = j * CH + pr * P + ho
                        nc.gpsimd.tensor_scalar_mul(
                            vpt, vnat[:, c0:c0 + DH], rec)
                        cur['vp'][(j, ho)] = vpt
                        if debug and pr == 0 and j == 0 and ho == 0:
                            nc.sync.dma_start(out=dbg["d_e00B"][:, :], in_=et)
                            nc.sync.dma_start(out=dbg["d_vp00"][:, :], in_=vpt)
                    if j >= 2:
                        for ho in (0, DH):
                            emit_av(pr, j - 2, ho, cur['y_ps'],
                                    cur['eB'][(j - 2, ho)],
                                    cur['vp'][(j - 2, ho)])
                        for ho in (0, DH):
                            cur['eB'].pop((j - 2, ho))
                    pacer.step(1)
                pacer.drain()

            # ---- tail: trailing av of last pair + output projection
            last = state[PAIRS - 1]
            for pj in (NT - 2, NT - 1):
                for ho in (0, DH):
                    emit_av(PAIRS - 1, pj, ho, last['y_ps'],
                            last['eB'][(pj, ho)], last['vp'][(pj, ho)])
            for hf in range(2):
                nc.vector.tensor_copy(
                    y_sb[PAIRS - 1][:, hf * HW:(hf + 1) * HW],
                    last['y_ps'][hf])

            for o in range(OUT // P):
                osb = p_osb.tile([P, N], f16, tag="osb", name="osb")
                for h in range(2):
                    ps = psY.tile([P, HW], f32, tag="Y", name=f"out{o}_{h}")
                    for c in range(CH // P):
                        for q in range(2):
                            ich = 2 * h + q
                            nc.tensor.matmul(
                                ps[:, q * 512:(q + 1) * 512],
                                lhsT=wout_sb[c][:, o * P:(o + 1) * P],
                                rhs=y_sb[c][:, ich * 512:(ich + 1) * 512],
                                start=(c == 0), stop=(c == CH // P - 1))
                    nc.vector.tensor_copy(osb[:, h * HW:(h + 1) * HW], ps)
                nc.sync.dma_start(out=outT[o * P:(o + 1) * P, :], in_=osb)
    return nc


def _shard_inputs(x, w_qkv, w_out, b_out):
    """Per-core inputs: core c -> (batch c//2, head-half c%2)."""
    in_maps = []
    for c in range(N_CORES):
        b, hh = c // 2, c % 2
        cols = slice(hh * CH, (hh + 1) * CH)
        xTc = np.ascontiguousarray(np.asarray(x[b]).T, dtype=np.float16)
        wq = w_qkv[:, 0 * F:1 * F][:, cols]
        wk = w_qkv[:, 1 * F:2 * F][:, cols]
        wv = w_qkv[:, 2 * F:3 * F][:, cols]
        wqkv_c = np.ascontiguousarray(
            np.concatenate([wq, wk, wv], axis=1), dtype=np.float16)
        wout_c = np.ascontiguousarray(w_out[cols, :], dtype=np.float16)
        in_maps.append({"xT": xTc, "wqkv": wqkv_c, "wout": wout_c})
    return in_maps


def _gather_outputs(results, b_out):
    out = np.empty((B, N, OUT), np.float32)
    for b in range(B):
        acc = (results[2 * b]["outT"].astype(np.float32)
               + results[2 * b + 1]["outT"].astype(np.float32))  # [OUT, N]
        out[b] = acc.T + b_out[None, :]
    return out


# Test instrumentation (harness just calls kernel(); these stay default).
_TRACE = False
_LAST_RESULT = None


def kernel(x, w_qkv, w_out, b_out):
    global _LAST_RESULT
    if os.environ.get("JAX_PLATFORMS") not in (None, "", "axon"):
        os.environ.pop("JAX_PLATFORMS", None)
    from concourse.bass_utils import run_bass_kernel_spmd

    nc = _build_nc()
    if not nc.is_finalized():
        nc.finalize()
    in_maps = _shard_inputs(np.asarray(x), np.asarray(w_qkv),
                            np.asarray(w_out), np.asarray(b_out))
    res = run_bass_kernel_spmd(nc, in_maps, list(range(N_CORES)),
                               trace=_TRACE)
    _LAST_RESULT = res
    return _gather_outputs(res.results, np.asarray(b_out, np.float32))


# ---------------------------------------------------------------------------
# Numpy emulation of the per-core device program (host-logic testing only).
def _emulate_core(m):
    xT, wqkv, wout = m["xT"], m["wqkv"], m["wout"]
    qT = (wqkv[:, 0:CH].T @ xT)          # [CH, N]
    kTm = (wqkv[:, CH:2 * CH].T @ xT)    # [CH, N]
    v = xT.T @ wqkv[:, 2 * CH:3 * CH]    # [N, CH]
    y = np.empty((CH, N), np.float32)
    for h in range(HH):
        qh = qT[h * DH:(h + 1) * DH, :]
        kh = kTm[h * DH:(h + 1) * DH, :]
        sT = kh.T @ qh                       # [j, i]
        e = np.exp(sT * SCALE)
        den = e.sum(axis=1, keepdims=True)   # over queries i, per key j
        vp = v[:, h * DH:(h + 1) * DH] / den
        y[h * DH:(h + 1) * DH, :] = vp.T @ e
    return (wout.T @ y).astype(np.float16)   # [OUT, N] f16 like device


def _kernel_emulated(x, w_qkv, w_out, b_out):
    in_maps = _shard_inputs(np.asarray(x), np.asarray(w_qkv),
                            np.asarray(w_out), np.asarray(b_out))
    results = [{"outT": _emulate_core(m)} for m in in_maps]
    return _gather_outputs(results, np.asarray(b_out, np.float32))


if __name__ == "__main__":
    # host-logic self-test vs reference math in float64-ish precision
    rng = np.random.default_rng(0)
    x = rng.standard_normal((B, N, F)).astype(np.float32)
    w_qkv = (rng.standard_normal((F, 3 * OUT)) / np.sqrt(F)).astype(np.float32)
    w_out = (rng.standard_normal((OUT, OUT)) / np.sqrt(OUT)).astype(np.float32)
    b_out = np.zeros((OUT,), np.float32)

    def ref(x, w_qkv, w_out, b_out):
        qkv = x @ w_qkv
        q, k, v = np.split(qkv, 3, axis=-1)

        def heads(t):
            return t.reshape(B, N, 16, DH).transpose(0, 2, 1, 3)
        q, k, v = heads(q), heads(k), heads(v)
        s = np.einsum('bhid,bhjd->bhij', q, k) * SCALE
        e = np.exp(s - s.max(axis=2, keepdims=True))
        a = e / e.sum(axis=2, keepdims=True)
        y = np.einsum('bhij,bhjd->bhid', a, v)
        y = y.transpose(0, 2, 1, 3).reshape(B, N, 16 * DH)
        return y @ w_out + b_out

    exp = ref(x, w_qkv, w_out, b_out)
    act = _kernel_emulated(x, w_qkv, w_out, b_out)
    rel = np.linalg.norm(act - exp) / np.linalg.norm(exp)
    print("emulated rel err:", rel)


# revision 30
# speedup vs baseline: 1.0569x; 1.0569x over previous
"""Trainium2 Bass kernel for MHA with query-axis softmax (nn_MHA_2568390443327).

Reference computation (B=4, N=2048, DIM=1024, 16 heads x 64):
    qkv = x @ w_qkv ; q,k,v = split(qkv)
    scores = (q @ k^T) * scale            # [b,h,i(query),j(key)]
    attn = softmax(scores, axis=QUERY)    # normalized over i, per key j
    y = attn @ v ; out = y @ w_out + b_out

Sharding (8 cores): batch (4) x head-half (2). Each core: its batch's x
(pre-transposed), qkv weight columns + w_out rows for its 8 heads, producing
a partial [OUT, N] f16 output. Host sums the two head-half partials per batch,
adds b_out, transposes back.

Per-core schedule: act (exp) engine is the bottleneck (~284us of [128,1024]
Exp tiles); everything else is laced around its cadence:
  - scores computed transposed S_T[j,i] so the query-axis softmax denominator
    is a free-axis reduce; exp tiles go to SBUF f16, denominators via DVE
    tensor_reduce (pass A) + reduce/add/recip (pass B), v-rescale on GpSimd.
  - attention runs per head-pair in two i-half passes (A: i 0..1023,
    B: 1024..2047); pass-A exp tiles are retained in SBUF so attn@v for both
    halves trails pass B by 2 slots, accumulating y in a 4-bank PSUM region.
  - the y PSUM region is idle during pass A; all projection matmuls (v, next
    pair's q/k, final out-proj) rotate through that same 4-bank pool, paced
    ~2 groups per act slot to keep the PE dense (HAM clock-gate stays 8/8).
  - scores ring: 2 x [128,1024] PSUM tiles (the other 4 banks).
All matmul operands f16 (full PE rate, fp32 PSUM accumulation).
"""

import os
import numpy as np

# ---------------------------------------------------------------------------
B = 4
N = 2048          # sequence length
F = 1024          # model dim (contraction for qkv proj)
DH = 64           # head dim
HH = 8            # heads per core
CH = HH * DH      # 512: per-core hidden
OUT = 1024        # output dim
SCALE = 0.125     # 1/sqrt(64)
N_CORES = 8

P = 128           # partitions
PAIRS = 4         # head pairs per core
NT = N // P       # 16 j-tiles
KT = F // P       # 8 k-tiles
HW = N // 2       # 1024: i-half width


def _build_nc():
    import concourse.bass as bass  # noqa: F401
    import concourse.mybir as mybir
    from concourse import bacc
    from concourse.tile import TileContext

    f32 = mybir.dt.float32
    f16 = mybir.dt.float16
    EXP = mybir.ActivationFunctionType.Exp
    AX = mybir.AxisListType.X
    ADD = mybir.AluOpType.add

    nc = bacc.Bacc(None, target_bir_lowering=False)

    xT = nc.declare_dram_parameter("xT", [F, N], f16, isOutput=False)
    wqkv = nc.declare_dram_parameter("wqkv", [F, 3 * CH], f16, isOutput=False)
    wout = nc.declare_dram_parameter("wout", [CH, OUT], f16, isOutput=False)
    outT = nc.declare_dram_parameter("outT", [OUT, N], f16, isOutput=True)

    debug = bool(os.environ.get("MHA_DEBUG"))
    dbg = {}
    if debug:
        for nm, shp, dt in [("d_q0", [P, N], f16), ("d_k0", [P, N], f16),
                            ("d_v", [P, NT * CH], f16), ("d_e00", [P, HW], f16),
                            ("d_e00B", [P, HW], f16), ("d_dA0", [P, NT], f16),
                            ("d_vp00", [P, DH], f16), ("d_y0", [P, N], f16)]:
            dbg[nm] = nc.declare_dram_parameter(nm, shp, dt, isOutput=True)

    with TileContext(nc) as tc:
        with (
            tc.tile_pool(name="p_x", bufs=1) as p_x,
            tc.tile_pool(name="p_w", bufs=1) as p_w,
            tc.tile_pool(name="p_wout", bufs=1) as p_wout,
            tc.tile_pool(name="p_qk", bufs=2) as p_qk,
            tc.tile_pool(name="p_v", bufs=1) as p_v,
            tc.tile_pool(name="p_eA", bufs=1) as p_eA,
            tc.tile_pool(name="p_eB", bufs=8) as p_eB,
            tc.tile_pool(name="p_scr", bufs=1) as p_scr,
            tc.tile_pool(name="p_ysb", bufs=1) as p_ysb,
            tc.tile_pool(name="p_dA", bufs=2) as p_dA,
            tc.tile_pool(name="p_sm", bufs=6) as p_sm,
            tc.tile_pool(name="p_vp", bufs=6) as p_vp,
            tc.tile_pool(name="p_osb", bufs=2) as p_osb,
            tc.tile_pool(name="psS", bufs=2, space="PSUM") as psS,
            tc.tile_pool(name="psY", bufs=2, space="PSUM") as psY,
        ):
            xt = [p_x.tile([P, N], f16, tag=f"x{k}", name=f"x{k}")
                  for k in range(KT)]
            wt = [p_w.tile([P, 3 * CH], f16, tag=f"w{k}", name=f"w{k}")
                  for k in range(KT)]
            wout_sb = [p_wout.tile([P, OUT], f16, tag=f"wo{c}", name=f"wo{c}")
                       for c in range(CH // P)]
            vnat = p_v.tile([P, NT * CH], f16, tag="v", name="vnat")
            # write-only sink for the den tensor_scalar+accum trick (the
            # accumulator is the real output; f16 streams keep DVE in 2x)
            scr = p_scr.tile([P, HW], f16, tag="scr", name="scr")
            MUL = mybir.AluOpType.mult

            def emit_den(dst_col, et):
                with nc.allow_low_precision("f16 den keeps DVE 2x"):
                    nc.vector.tensor_scalar(scr, et, 1.0, 0.0, MUL, ADD,
                                            accum_out=dst_col)
            y_sb = [p_ysb.tile([P, N], f16, tag=f"y{p_}", name=f"y{p_}")
                    for p_ in range(PAIRS)]

            for k in range(KT):
                nc.sync.dma_start(out=xt[k], in_=xT[k * P:(k + 1) * P, :])
                nc.sync.dma_start(out=wt[k], in_=wqkv[k * P:(k + 1) * P, :])
            for c in range(CH // P):
                nc.sync.dma_start(out=wout_sb[c],
                                  in_=wout[c * P:(c + 1) * P, :])

            # ---------------- projection helpers (psY rotations) ----------
            qk_t = {}    # pair -> (qT tile, kT tile)

            def qk_rot_units(pr):
                """q/k projection for a pair as psY [128,1024] rotations.
                Order (q-h0, k-h0, q-h1, k-h1): pass-A scores need only the
                h0 halves of q plus k, so attention starts 2 rotations in."""
                dsts = {}
                for sec in (0, 1):
                    dsts[sec] = p_qk.tile([P, N], f16,
                                          tag="q" if sec == 0 else "k",
                                          name=f"{'qk'[sec]}T{pr}")
                    qk_t.setdefault(pr, {})[sec] = dsts[sec]
                for h in range(2):
                    for sec in (0, 1):
                        dst = dsts[sec]
                        st = {}

                        def alloc(sec=sec, h=h):
                            st['ps'] = psY.tile([P, HW], f32, tag="Y",
                                                name=f"qk{pr}_{sec}_{h}")

                        def grp(nch, sec=sec):
                            ps = st['ps']
                            for k in range(KT):
                                nc.tensor.matmul(
                                    ps[:, (nch % 2) * 512:(nch % 2 + 1) * 512],
                                    lhsT=wt[k][:, sec * CH + pr * P:
                                               sec * CH + (pr + 1) * P],
                                    rhs=xt[k][:, nch * 512:(nch + 1) * 512],
                                    start=(k == 0), stop=(k == KT - 1))

                        def cast(dst=dst, h=h):
                            nc.vector.tensor_copy(
                                dst[:, h * HW:(h + 1) * HW], st['ps'])

                        yield ('alloc', alloc)
                        for nch in (2 * h, 2 * h + 1):
                            yield ('group', lambda nch=nch, g=grp: g(nch))
                        yield ('cast', cast)

            def v_rot_units():
                for vbase in range(0, NT, 2):
                    st = {}

                    def alloc(vbase=vbase):
                        st['ps'] = psY.tile([P, HW], f32, tag="Y",
                                            name=f"v{vbase}")

                    def grp(q, vbase=vbase):
                        j = vbase + q
                        ps = st['ps']
                        for k in range(KT):
                            nc.tensor.matmul(
                                ps[:, q * 512:(q + 1) * 512],
                                lhsT=xt[k][:, j * P:(j + 1) * P],
                                rhs=wt[k][:, 2 * CH:3 * CH],
                                start=(k == 0), stop=(k == KT - 1))

                    def cast(vbase=vbase):
                        nc.vector.tensor_copy(
                            vnat[:, vbase * CH:(vbase + 2) * CH], st['ps'])

                    yield ('alloc', alloc)
                    for q in range(2):
                        yield ('group', lambda q=q, g=grp: g(q))
                    yield ('cast', cast)

            def run_units(units):
                """Emit all units of a projection immediately."""
                for kind, fn in units:
                    fn()

            class Pacer:
                """Paced emission of projection units into attention slots."""
                def __init__(self):
                    self.units = []
                    self.i = 0

                def extend(self, gen):
                    self.units.extend(gen)

                def step(self, ngroups):
                    """Emit until `ngroups` matmul groups are emitted."""
                    g = 0
                    while self.i < len(self.units) and g < ngroups:
                        kind, fn = self.units[self.i]
                        fn()
                        self.i += 1
                        if kind == 'group':
                            g += 1

                def drain(self):
                    while self.i < len(self.units):
                        self.units[self.i][1]()
                        self.i += 1

            # ---------------- attention ----------------------------------
            # lead-in: only q-h0 + k-h0 of pair 0 (8 units); the h1
            # rotations go to the front of pair 0's pacer queue
            units0 = list(qk_rot_units(0))
            run_units(units0[:8])

            eA = {}      # (j, ho) -> pass-A exp tile (per-pair reuse)
            state = {}   # per (pr): denA tiles, rec/vp handles

            def emit_scores(pr, j, half, ho):
                sps = psS.tile([P, HW], f32, tag="S",
                               name=f"s{pr}_{j}_{half}_{ho}")
                qt = qk_t[pr][0]
                kt = qk_t[pr][1]
                for c2 in range(2):
                    nc.tensor.matmul(
                        sps[:, c2 * 512:(c2 + 1) * 512],
                        lhsT=kt[ho:ho + DH, j * P:(j + 1) * P],
                        rhs=qt[ho:ho + DH,
                               half * HW + c2 * 512:half * HW + (c2 + 1) * 512],
                        start=True, stop=True, tile_position=(ho, 0))
                return sps

            def emit_act(pr, j, half, ho, sps):
                if half == 0:
                    et = p_eA.tile([P, HW], f16, tag=f"eA{j}_{ho}",
                                   name=f"eA{j}_{ho}")
                    eA[(j, ho)] = et
                else:
                    et = p_eB.tile([P, HW], f16, tag="eB", name="eB")
                nc.scalar.activation(et, sps, EXP, scale=SCALE)
                return et

            def emit_av(pr, j, ho, y_ps, eBt, vpt):
                for hf, et in ((0, eA[(j, ho)]), (1, eBt)):
                    for c2 in range(2):
                        nc.tensor.matmul(
                            y_ps[hf][ho:ho + DH, c2 * 512:(c2 + 1) * 512],
                            lhsT=vpt,
                            rhs=et[:, c2 * 512:(c2 + 1) * 512],
                            start=(j == 0), stop=(j == NT - 1),
                            tile_position=(0, ho), skip_group_check=True)

            for pr in range(PAIRS):
                pacer = Pacer()
                if pr == 0:
                    pacer.extend(units0[8:])
                    pacer.extend(v_rot_units())
                    pacer.extend(qk_rot_units(1))
                elif pr < PAIRS - 1:
                    pacer.extend(qk_rot_units(pr + 1))

                dA = {ho: p_dA.tile([P, NT], f16, tag=f"dA{ho}",
                                    name=f"dA{pr}_{ho}")
                      for ho in (0, DH)}
                prev = state.get(pr - 1)

                # ---- pass A: i in [0, 1024) : scores + exp + denA
                for j in range(NT):
                    for ho in (0, DH):
                        sps = emit_scores(pr, j, 0, ho)
                        et = emit_act(pr, j, 0, ho, sps)
                        emit_den(dA[ho][:, j:j + 1], et)
                    # trailing work of the previous pair goes right after
                    # j=0's scores so the act engine never waits at the
                    # pair boundary
                    if prev is not None and j == 0:
                        for pj in (NT - 3, NT - 2, NT - 1):
                            for ho in (0, DH):
                                emit_av(pr - 1, pj, ho, prev['y_ps'],
                                        prev['eB'][(pj, ho)],
                                        prev['vp'][(pj, ho)])
                        for hf in range(2):
                            nc.vector.tensor_copy(
                                y_sb[pr - 1][:, hf * HW:(hf + 1) * HW],
                                prev['y_ps'][hf])
                        if debug and pr == 1:
                            nc.sync.dma_start(out=dbg["d_y0"][:, :],
                                              in_=y_sb[0])
                        state.pop(pr - 1)
                    if debug and pr == 0 and j == 0:
                        nc.sync.dma_start(out=dbg["d_e00"][:, :], in_=eA[(0, 0)])
                    pacer.step(2 if pr == 0 else 1)

                # ---- pass B: i in [1024, 2048) : + den total + av(j-2)
                if debug and pr == 0:
                    nc.sync.dma_start(out=dbg["d_q0"][:, :], in_=qk_t[0][0])
                    nc.sync.dma_start(out=dbg["d_k0"][:, :], in_=qk_t[0][1])
                    nc.sync.dma_start(out=dbg["d_v"][:, :], in_=vnat)
                    nc.sync.dma_start(out=dbg["d_dA0"][:, :], in_=dA[0])
                cur = {'eB': {}, 'vp': {},
                       'y_ps': [psY.tile([P, HW], f32, tag="Y",
                                         name=f"yps{pr}_{hf}")
                                for hf in range(2)]}
                state[pr] = cur
                for j in range(NT):
                    for ho in (0, DH):
                        sps = emit_scores(pr, j, 1, ho)
                        et = emit_act(pr, j, 1, ho, sps)
                        cur['eB'][(j, ho)] = et
                        dB = p_sm.tile([P, 1], f16, tag="dB", name="dB")
                        emit_den(dB, et)
                        dtot = p_sm.tile([P, 1], f32, tag="dt", name="dt")
                        nc.vector.tensor_add(dtot, dA[ho][:, j:j + 1], dB)
                        rec = p_sm.tile([P, 1], f32, tag="rc", name="rc")
                        nc.vector.reciprocal(rec, dtot)
                        vpt = p_vp.tile([P, DH], f16, tag="vp", name="vp")
                        c0 = j * CH + pr * P + ho
                        nc.gpsimd.tensor_scalar_mul(
                            vpt, vnat[:, c0:c0 + DH], rec)
                        cur['vp'][(j, ho)] = vpt
                        if debug and pr == 0 and j == 0 and ho == 0:
                            nc.sync.dma_start(out=dbg["d_e00B"][:, :], in_=et)
                            nc.sync.dma_start(out=dbg["d_vp00"][:, :], in_=vpt)
                    if j >= 3:
                        for ho in (0, DH):
                            emit_av(pr, j - 3, ho, cur['y_ps'],
                                    cur['eB'][(j - 3, ho)],
                                    cur['vp'][(j - 3, ho)])
                        for ho in (0, DH):
                            cur['eB'].pop((j - 3, ho))
                    pacer.step(1)
                pacer.drain()

            # ---- tail: trailing av of last pair + output projection
            last = state[PAIRS - 1]
            for pj in (NT - 3, NT - 2, NT - 1):
                for ho in (0, DH):
                    emit_av(PAIRS - 1, pj, ho, last['y_ps'],
                            last['eB'][(pj, ho)], last['vp'][(pj, ho)])
            for hf in range(2):
                nc.vector.tensor_copy(
                    y_sb[PAIRS - 1][:, hf * HW:(hf + 1) * HW],
                    last['y_ps'][hf])

            for o in range(OUT // P):
                osb = p_osb.tile([P, N], f16, tag="osb", name="osb")
                for h in range(2):
                    ps = psY.tile([P, HW], f32, tag="Y", name=f"out{o}_{h}")
                    for c in range(CH // P):
                        for q in range(2):
                            ich = 2 * h + q
                            nc.tensor.matmul(
                                ps[:, q * 512:(q + 1) * 512],
                                lhsT=wout_sb[c][:, o * P:(o + 1) * P],
                                rhs=y_sb[c][:, ich * 512:(ich + 1) * 512],
                                start=(c == 0), stop=(c == CH // P - 1))
                    nc.vector.tensor_copy(osb[:, h * HW:(h + 1) * HW], ps)
                nc.sync.dma_start(out=outT[o * P:(o + 1) * P, :], in_=osb)
    return nc


def _shard_inputs(x, w_qkv, w_out, b_out):
    """Per-core inputs: core c -> (batch c//2, head-half c%2)."""
    in_maps = []
    for c in range(N_CORES):
        b, hh = c // 2, c % 2
        cols = slice(hh * CH, (hh + 1) * CH)
        xTc = np.ascontiguousarray(np.asarray(x[b]).T, dtype=np.float16)
        wq = w_qkv[:, 0 * F:1 * F][:, cols]
        wk = w_qkv[:, 1 * F:2 * F][:, cols]
        wv = w_qkv[:, 2 * F:3 * F][:, cols]
        wqkv_c = np.ascontiguousarray(
            np.concatenate([wq, wk, wv], axis=1), dtype=np.float16)
        wout_c = np.ascontiguousarray(w_out[cols, :], dtype=np.float16)
        in_maps.append({"xT": xTc, "wqkv": wqkv_c, "wout": wout_c})
    return in_maps


def _gather_outputs(results, b_out):
    out = np.empty((B, N, OUT), np.float32)
    for b in range(B):
        acc = (results[2 * b]["outT"].astype(np.float32)
               + results[2 * b + 1]["outT"].astype(np.float32))  # [OUT, N]
        out[b] = acc.T + b_out[None, :]
    return out


# Test instrumentation (harness just calls kernel(); these stay default).
_TRACE = False
_LAST_RESULT = None


def kernel(x, w_qkv, w_out, b_out):
    global _LAST_RESULT
    if os.environ.get("JAX_PLATFORMS") not in (None, "", "axon"):
        os.environ.pop("JAX_PLATFORMS", None)
    from concourse.bass_utils import run_bass_kernel_spmd

    nc = _build_nc()
    if not nc.is_finalized():
        nc.finalize()
    in_maps = _shard_inputs(np.asarray(x), np.asarray(w_qkv),
                            np.asarray(w_out), np.asarray(b_out))
    res = run_bass_kernel_spmd(nc, in_maps, list(range(N_CORES)),
                               trace=_TRACE)
    _LAST_RESULT = res
    return _gather_outputs(res.results, np.asarray(b_out, np.float32))


# ---------------------------------------------------------------------------
# Numpy emulation of the per-core device program (host-logic testing only).
def _emulate_core(m):
    xT, wqkv, wout = m["xT"], m["wqkv"], m["wout"]
    qT = (wqkv[:, 0:CH].T @ xT)          # [CH, N]
    kTm = (wqkv[:, CH:2 * CH].T @ xT)    # [CH, N]
    v = xT.T @ wqkv[:, 2 * CH:3 * CH]    # [N, CH]
    y = np.empty((CH, N), np.float32)
    for h in range(HH):
        qh = qT[h * DH:(h + 1) * DH, :]
        kh = kTm[h * DH:(h + 1) * DH, :]
        sT = kh.T @ qh                       # [j, i]
        e = np.exp(sT * SCALE)
        den = e.sum(axis=1, keepdims=True)   # over queries i, per key j
        vp = v[:, h * DH:(h + 1) * DH] / den
        y[h * DH:(h + 1) * DH, :] = vp.T @ e
    return (wout.T @ y).astype(np.float16)   # [OUT, N] f16 like device


def _kernel_emulated(x, w_qkv, w_out, b_out):
    in_maps = _shard_inputs(np.asarray(x), np.asarray(w_qkv),
                            np.asarray(w_out), np.asarray(b_out))
    results = [{"outT": _emulate_core(m)} for m in in_maps]
    return _gather_outputs(results, np.asarray(b_out, np.float32))


if __name__ == "__main__":
    # host-logic self-test vs reference math in float64-ish precision
    rng = np.random.default_rng(0)
    x = rng.standard_normal((B, N, F)).astype(np.float32)
    w_qkv = (rng.standard_normal((F, 3 * OUT)) / np.sqrt(F)).astype(np.float32)
    w_out = (rng.standard_normal((OUT, OUT)) / np.sqrt(OUT)).astype(np.float32)
    b_out = np.zeros((OUT,), np.float32)

    def ref(x, w_qkv, w_out, b_out):
        qkv = x @ w_qkv
        q, k, v = np.split(qkv, 3, axis=-1)

        def heads(t):
            return t.reshape(B, N, 16, DH).transpose(0, 2, 1, 3)
        q, k, v = heads(q), heads(k), heads(v)
        s = np.einsum('bhid,bhjd->bhij', q, k) * SCALE
        e = np.exp(s - s.max(axis=2, keepdims=True))
        a = e / e.sum(axis=2, keepdims=True)
        y = np.einsum('bhij,bhjd->bhid', a, v)
        y = y.transpose(0, 2, 1, 3).reshape(B, N, 16 * DH)
        return y @ w_out + b_out

    exp = ref(x, w_qkv, w_out, b_out)
    act = _kernel_emulated(x, w_qkv, w_out, b_out)
    rel = np.linalg.norm(act - exp) / np.linalg.norm(exp)
    print("emulated rel err:", rel)


# revision 31
# speedup vs baseline: 1.1917x; 1.1275x over previous
"""Trainium2 Bass kernel for MHA with query-axis softmax (nn_MHA_2568390443327).

Reference computation (B=4, N=2048, DIM=1024, 16 heads x 64):
    qkv = x @ w_qkv ; q,k,v = split(qkv)
    scores = (q @ k^T) * scale            # [b,h,i(query),j(key)]
    attn = softmax(scores, axis=QUERY)    # normalized over i, per key j
    y = attn @ v ; out = y @ w_out + b_out

Sharding (8 cores): batch (4) x head-half (2). Each core: its batch's x
(pre-transposed), qkv weight columns + w_out rows for its 8 heads, producing
a partial [OUT, N] f16 output. Host sums the two head-half partials per batch,
adds b_out, transposes back.

Per-core schedule: act (exp) engine is the bottleneck (~284us of [128,1024]
Exp tiles); everything else is laced around its cadence:
  - scores computed transposed S_T[j,i] so the query-axis softmax denominator
    is a free-axis reduce; exp tiles go to SBUF f16, denominators via DVE
    tensor_reduce (pass A) + reduce/add/recip (pass B), v-rescale on GpSimd.
  - attention runs per head-pair in two i-half passes (A: i 0..1023,
    B: 1024..2047); pass-A exp tiles are retained in SBUF so attn@v for both
    halves trails pass B by 2 slots, accumulating y in a 4-bank PSUM region.
  - the y PSUM region is idle during pass A; all projection matmuls (v, next
    pair's q/k, final out-proj) rotate through that same 4-bank pool, paced
    ~2 groups per act slot to keep the PE dense (HAM clock-gate stays 8/8).
  - scores ring: 2 x [128,1024] PSUM tiles (the other 4 banks).
All matmul operands f16 (full PE rate, fp32 PSUM accumulation).
"""

import os
import numpy as np

# ---------------------------------------------------------------------------
B = 4
N = 2048          # sequence length
F = 1024          # model dim (contraction for qkv proj)
DH = 64           # head dim
HH = 8            # heads per core
CH = HH * DH      # 512: per-core hidden
OUT = 1024        # output dim
SCALE = 0.125     # 1/sqrt(64)
N_CORES = 8

P = 128           # partitions
PAIRS = 4         # head pairs per core
NT = N // P       # 16 j-tiles
KT = F // P       # 8 k-tiles
HW = N // 2       # 1024: i-half width


def _build_nc():
    import concourse.bass as bass  # noqa: F401
    import concourse.mybir as mybir
    from concourse import bacc
    from concourse.tile import TileContext

    f32 = mybir.dt.float32
    f16 = mybir.dt.float16
    EXP = mybir.ActivationFunctionType.Exp
    AX = mybir.AxisListType.X
    ADD = mybir.AluOpType.add

    nc = bacc.Bacc(None, target_bir_lowering=False)

    xT = nc.declare_dram_parameter("xT", [F, N], f16, isOutput=False)
    wqkv = nc.declare_dram_parameter("wqkv", [F, 3 * CH], f16, isOutput=False)
    wout = nc.declare_dram_parameter("wout", [CH, OUT], f16, isOutput=False)
    outT = nc.declare_dram_parameter("outT", [OUT, N], f16, isOutput=True)

    debug = bool(os.environ.get("MHA_DEBUG"))
    dbg = {}
    if debug:
        for nm, shp, dt in [("d_q0", [P, N], f16), ("d_k0", [P, N], f16),
                            ("d_v", [P, NT * CH], f16), ("d_e00", [P, HW], f16),
                            ("d_e00B", [P, HW], f16), ("d_dA0", [P, NT], f16),
                            ("d_vp00", [P, DH], f16), ("d_y0", [P, N], f16)]:
            dbg[nm] = nc.declare_dram_parameter(nm, shp, dt, isOutput=True)

    with TileContext(nc) as tc:
        with (
            tc.tile_pool(name="p_x", bufs=1) as p_x,
            tc.tile_pool(name="p_w", bufs=1) as p_w,
            tc.tile_pool(name="p_wout", bufs=1) as p_wout,
            tc.tile_pool(name="p_qk", bufs=2) as p_qk,
            tc.tile_pool(name="p_v", bufs=1) as p_v,
            tc.tile_pool(name="p_eA", bufs=1) as p_eA,
            tc.tile_pool(name="p_eB", bufs=8) as p_eB,
            tc.tile_pool(name="p_scr", bufs=1) as p_scr,
            tc.tile_pool(name="p_ysb", bufs=1) as p_ysb,
            tc.tile_pool(name="p_dA", bufs=2) as p_dA,
            tc.tile_pool(name="p_sm", bufs=6) as p_sm,
            tc.tile_pool(name="p_vp", bufs=6) as p_vp,
            tc.tile_pool(name="p_osb", bufs=2) as p_osb,
            tc.tile_pool(name="psS", bufs=2, space="PSUM") as psS,
            tc.tile_pool(name="psY", bufs=2, space="PSUM") as psY,
        ):
            xt = [p_x.tile([P, N], f16, tag=f"x{k}", name=f"x{k}")
                  for k in range(KT)]
            wt = [p_w.tile([P, 3 * CH], f16, tag=f"w{k}", name=f"w{k}")
                  for k in range(KT)]
            wout_sb = [p_wout.tile([P, OUT], f16, tag=f"wo{c}", name=f"wo{c}")
                       for c in range(CH // P)]
            vnat = p_v.tile([P, NT * CH], f16, tag="v", name="vnat")
            # write-only sink for the den tensor_scalar+accum trick (the
            # accumulator is the real output; f16 streams keep DVE in 2x)
            scr = p_scr.tile([P, HW], f16, tag="scr", name="scr")
            MUL = mybir.AluOpType.mult

            def emit_den(dst_col, et):
                with nc.allow_low_precision("f16 den keeps DVE 2x"):
                    nc.vector.tensor_scalar(scr, et, 1.0, 0.0, MUL, ADD,
                                            accum_out=dst_col)
            y_sb = [p_ysb.tile([P, N], f16, tag=f"y{p_}", name=f"y{p_}")
                    for p_ in range(PAIRS)]

            # wave 1: what the lead-in q/k rotations touch first
            for k in range(KT):
                nc.sync.dma_start(out=xt[k][:, 0:HW],
                                  in_=xT[k * P:(k + 1) * P, 0:HW])
                nc.sync.dma_start(out=wt[k][:, 0:2 * CH],
                                  in_=wqkv[k * P:(k + 1) * P, 0:2 * CH])
            # wave 2: the rest
            for k in range(KT):
                nc.sync.dma_start(out=xt[k][:, HW:N],
                                  in_=xT[k * P:(k + 1) * P, HW:N])
                nc.sync.dma_start(out=wt[k][:, 2 * CH:3 * CH],
                                  in_=wqkv[k * P:(k + 1) * P, 2 * CH:3 * CH])
            for c in range(CH // P):
                nc.sync.dma_start(out=wout_sb[c],
                                  in_=wout[c * P:(c + 1) * P, :])

            # ---------------- projection helpers (psY rotations) ----------
            qk_t = {}    # pair -> (qT tile, kT tile)

            def qk_rot_units(pr):
                """q/k projection for a pair as psY [128,1024] rotations.
                Order (q-h0, k-h0, q-h1, k-h1): pass-A scores need only the
                h0 halves of q plus k, so attention starts 2 rotations in."""
                dsts = {}
                for sec in (0, 1):
                    dsts[sec] = p_qk.tile([P, N], f16,
                                          tag="q" if sec == 0 else "k",
                                          name=f"{'qk'[sec]}T{pr}")
                    qk_t.setdefault(pr, {})[sec] = dsts[sec]
                for h in range(2):
                    for sec in (0, 1):
                        dst = dsts[sec]
                        st = {}

                        def alloc(sec=sec, h=h):
                            st['ps'] = psY.tile([P, HW], f32, tag="Y",
                                                name=f"qk{pr}_{sec}_{h}")

                        def grp(nch, sec=sec):
                            ps = st['ps']
                            for k in range(KT):
                                nc.tensor.matmul(
                                    ps[:, (nch % 2) * 512:(nch % 2 + 1) * 512],
                                    lhsT=wt[k][:, sec * CH + pr * P:
                                               sec * CH + (pr + 1) * P],
                                    rhs=xt[k][:, nch * 512:(nch + 1) * 512],
                                    start=(k == 0), stop=(k == KT - 1))

                        def cast(dst=dst, h=h):
                            nc.vector.tensor_copy(
                                dst[:, h * HW:(h + 1) * HW], st['ps'])

                        yield ('alloc', alloc)
                        for nch in (2 * h, 2 * h + 1):
                            yield ('group', lambda nch=nch, g=grp: g(nch))
                        yield ('cast', cast)

            def v_rot_units():
                for vbase in range(0, NT, 2):
                    st = {}

                    def alloc(vbase=vbase):
                        st['ps'] = psY.tile([P, HW], f32, tag="Y",
                                            name=f"v{vbase}")

                    def grp(q, vbase=vbase):
                        j = vbase + q
                        ps = st['ps']
                        for k in range(KT):
                            nc.tensor.matmul(
                                ps[:, q * 512:(q + 1) * 512],
                                lhsT=xt[k][:, j * P:(j + 1) * P],
                                rhs=wt[k][:, 2 * CH:3 * CH],
                                start=(k == 0), stop=(k == KT - 1))

                    def cast(vbase=vbase):
                        nc.vector.tensor_copy(
                            vnat[:, vbase * CH:(vbase + 2) * CH], st['ps'])

                    yield ('alloc', alloc)
                    for q in range(2):
                        yield ('group', lambda q=q, g=grp: g(q))
                    yield ('cast', cast)

            def run_units(units):
                """Emit all units of a projection immediately."""
                for kind, fn in units:
                    fn()

            class Pacer:
                """Paced emission of projection units into attention slots."""
                def __init__(self):
                    self.units = []
                    self.i = 0
                    self.groups_total = 0
                    self.groups_done = 0

                def extend(self, gen):
                    for u in gen:
                        self.units.append(u)
                        if u[0] == 'group':
                            self.groups_total += 1

                def step(self, ngroups, reserve=0):
                    """Emit up to `ngroups` matmul groups, keeping
                    `reserve` groups for later slots; always flush a
                    trailing cast."""
                    g = 0
                    while (self.i < len(self.units) and g < ngroups
                           and self.groups_total - self.groups_done > reserve):
                        kind, fn = self.units[self.i]
                        fn()
                        self.i += 1
                        if kind == 'group':
                            self.groups_done += 1
                            g += 1
                    while (self.i < len(self.units)
                           and self.units[self.i][0] == 'cast'):
                        self.units[self.i][1]()
                        self.i += 1

                def drain(self):
                    while self.i < len(self.units):
                        self.units[self.i][1]()
                        self.i += 1

            # ---------------- attention ----------------------------------
            # lead-in: only q-h0 + k-h0 of pair 0 (8 units); the h1
            # rotations go to the front of pair 0's pacer queue
            units0 = list(qk_rot_units(0))
            run_units(units0[:8])

            eA = {}      # (j, ho) -> pass-A exp tile (per-pair reuse)
            state = {}   # per (pr): denA tiles, rec/vp handles

            def emit_scores(pr, j, half, ho):
                sps = psS.tile([P, HW], f32, tag="S",
                               name=f"s{pr}_{j}_{half}_{ho}")
                qt = qk_t[pr][0]
                kt = qk_t[pr][1]
                for c2 in range(2):
                    nc.tensor.matmul(
                        sps[:, c2 * 512:(c2 + 1) * 512],
                        lhsT=kt[ho:ho + DH, j * P:(j + 1) * P],
                        rhs=qt[ho:ho + DH,
                               half * HW + c2 * 512:half * HW + (c2 + 1) * 512],
                        start=True, stop=True, tile_position=(ho, 0))
                return sps

            def emit_act(pr, j, half, ho, sps, accum=None):
                if half == 0:
                    et = p_eA.tile([P, HW], f16, tag=f"eA{j}_{ho}",
                                   name=f"eA{j}_{ho}")
                    eA[(j, ho)] = et
                else:
                    et = p_eB.tile([P, HW], f16, tag="eB", name="eB")
                nc.scalar.activation(et, sps, EXP, scale=SCALE,
                                     accum_out=accum)
                return et

            def emit_av(pr, j, ho, y_ps, eBt, vpt):
                for hf, et in ((0, eA[(j, ho)]), (1, eBt)):
                    for c2 in range(2):
                        nc.tensor.matmul(
                            y_ps[hf][ho:ho + DH, c2 * 512:(c2 + 1) * 512],
                            lhsT=vpt,
                            rhs=et[:, c2 * 512:(c2 + 1) * 512],
                            start=(j == 0), stop=(j == NT - 1),
                            tile_position=(0, ho), skip_group_check=True)

            for pr in range(PAIRS):
                pacer = Pacer()
                if pr == 0:
                    pacer.extend(units0[8:])
                    pacer.extend(v_rot_units())
                    pacer.extend(qk_rot_units(1))
                elif pr < PAIRS - 1:
                    pacer.extend(qk_rot_units(pr + 1))

                dA = {ho: p_dA.tile([P, NT], f16, tag=f"dA{ho}",
                                    name=f"dA{pr}_{ho}")
                      for ho in (0, DH)}
                prev = state.get(pr - 1)

                # ---- pass A: i in [0, 1024) : scores + exp + denA
                for j in range(NT):
                    for ho in (0, DH):
                        sps = emit_scores(pr, j, 0, ho)
                        et = emit_act(pr, j, 0, ho, sps)
                        emit_den(dA[ho][:, j:j + 1], et)
                    # trailing work of the previous pair goes right after
                    # j=0's scores so the act engine never waits at the
                    # pair boundary
                    if prev is not None and j == 0:
                        for pj in (NT - 2, NT - 1):
                            for ho in (0, DH):
                                emit_av(pr - 1, pj, ho, prev['y_ps'],
                                        prev['eB'][(pj, ho)],
                                        prev['vp'][(pj, ho)])
                        for hf in range(2):
                            nc.vector.tensor_copy(
                                y_sb[pr - 1][:, hf * HW:(hf + 1) * HW],
                                prev['y_ps'][hf])
                        if debug and pr == 1:
                            nc.sync.dma_start(out=dbg["d_y0"][:, :],
                                              in_=y_sb[0])
                        state.pop(pr - 1)
                    if debug and pr == 0 and j == 0:
                        nc.sync.dma_start(out=dbg["d_e00"][:, :], in_=eA[(0, 0)])
                    pacer.step(2 if pr == 0 else 1, reserve=3)

                # ---- pass B: i in [1024, 2048) : + den total + av(j-2)
                if debug and pr == 0:
                    nc.sync.dma_start(out=dbg["d_q0"][:, :], in_=qk_t[0][0])
                    nc.sync.dma_start(out=dbg["d_k0"][:, :], in_=qk_t[0][1])
                    nc.sync.dma_start(out=dbg["d_v"][:, :], in_=vnat)
                    nc.sync.dma_start(out=dbg["d_dA0"][:, :], in_=dA[0])
                cur = {'eB': {}, 'vp': {},
                       'y_ps': [psY.tile([P, HW], f32, tag="Y",
                                         name=f"yps{pr}_{hf}")
                                for hf in range(2)]}
                state[pr] = cur
                for j in range(NT):
                    for ho in (0, DH):
                        sps = emit_scores(pr, j, 1, ho)
                        dB = p_sm.tile([P, 1], f32, tag="dB", name="dB")
                        et = emit_act(pr, j, 1, ho, sps, accum=dB)
                        cur['eB'][(j, ho)] = et
                        dtot = p_sm.tile([P, 1], f32, tag="dt", name="dt")
                        nc.vector.tensor_add(dtot, dA[ho][:, j:j + 1], dB)
                        rec = p_sm.tile([P, 1], f32, tag="rc", name="rc")
                        nc.vector.reciprocal(rec, dtot)
                        vpt = p_vp.tile([P, DH], f16, tag="vp", name="vp")
                        c0 = j * CH + pr * P + ho
                        nc.gpsimd.tensor_scalar_mul(
                            vpt, vnat[:, c0:c0 + DH], rec)
                        cur['vp'][(j, ho)] = vpt
                        if debug and pr == 0 and j == 0 and ho == 0:
                            nc.sync.dma_start(out=dbg["d_e00B"][:, :], in_=et)
                            nc.sync.dma_start(out=dbg["d_vp00"][:, :], in_=vpt)
                    if j >= 2:
                        for ho in (0, DH):
                            emit_av(pr, j - 2, ho, cur['y_ps'],
                                    cur['eB'][(j - 2, ho)],
                                    cur['vp'][(j - 2, ho)])
                        for ho in (0, DH):
                            cur['eB'].pop((j - 2, ho))
                    pacer.step(2)
                pacer.drain()

            # ---- tail: trailing av of last pair + output projection
            last = state[PAIRS - 1]
            for pj in (NT - 2, NT - 1):
                for ho in (0, DH):
                    emit_av(PAIRS - 1, pj, ho, last['y_ps'],
                            last['eB'][(pj, ho)], last['vp'][(pj, ho)])
            for hf in range(2):
                nc.vector.tensor_copy(
                    y_sb[PAIRS - 1][:, hf * HW:(hf + 1) * HW],
                    last['y_ps'][hf])

            for o in range(OUT // P):
                osb = p_osb.tile([P, N], f16, tag="osb", name="osb")
                for h in range(2):
                    ps = psY.tile([P, HW], f32, tag="Y", name=f"out{o}_{h}")
                    for c in range(CH // P):
                        for q in range(2):
                            ich = 2 * h + q
                            nc.tensor.matmul(
                                ps[:, q * 512:(q + 1) * 512],
                                lhsT=wout_sb[c][:, o * P:(o + 1) * P],
                                rhs=y_sb[c][:, ich * 512:(ich + 1) * 512],
                                start=(c == 0), stop=(c == CH // P - 1))
                    nc.vector.tensor_copy(osb[:, h * HW:(h + 1) * HW], ps)
                nc.sync.dma_start(out=outT[o * P:(o + 1) * P, :], in_=osb)
    return nc


def _shard_inputs(x, w_qkv, w_out, b_out):
    """Per-core inputs: core c -> (batch c//2, head-half c%2)."""
    in_maps = []
    for c in range(N_CORES):
        b, hh = c // 2, c % 2
        cols = slice(hh * CH, (hh + 1) * CH)
        xTc = np.ascontiguousarray(np.asarray(x[b]).T, dtype=np.float16)
        wq = w_qkv[:, 0 * F:1 * F][:, cols]
        wk = w_qkv[:, 1 * F:2 * F][:, cols]
        wv = w_qkv[:, 2 * F:3 * F][:, cols]
        wqkv_c = np.ascontiguousarray(
            np.concatenate([wq, wk, wv], axis=1), dtype=np.float16)
        wout_c = np.ascontiguousarray(w_out[cols, :], dtype=np.float16)
        in_maps.append({"xT": xTc, "wqkv": wqkv_c, "wout": wout_c})
    return in_maps


def _gather_outputs(results, b_out):
    out = np.empty((B, N, OUT), np.float32)
    for b in range(B):
        acc = (results[2 * b]["outT"].astype(np.float32)
               + results[2 * b + 1]["outT"].astype(np.float32))  # [OUT, N]
        out[b] = acc.T + b_out[None, :]
    return out


# Test instrumentation (harness just calls kernel(); these stay default).
_TRACE = False
_LAST_RESULT = None


def kernel(x, w_qkv, w_out, b_out):
    global _LAST_RESULT
    if os.environ.get("JAX_PLATFORMS") not in (None, "", "axon"):
        os.environ.pop("JAX_PLATFORMS", None)
    from concourse.bass_utils import run_bass_kernel_spmd

    nc = _build_nc()
    if not nc.is_finalized():
        nc.finalize()
    in_maps = _shard_inputs(np.asarray(x), np.asarray(w_qkv),
                            np.asarray(w_out), np.asarray(b_out))
    res = run_bass_kernel_spmd(nc, in_maps, list(range(N_CORES)),
                               trace=_TRACE)
    _LAST_RESULT = res
    return _gather_outputs(res.results, np.asarray(b_out, np.float32))


# ---------------------------------------------------------------------------
# Numpy emulation of the per-core device program (host-logic testing only).
def _emulate_core(m):
    xT, wqkv, wout = m["xT"], m["wqkv"], m["wout"]
    qT = (wqkv[:, 0:CH].T @ xT)          # [CH, N]
    kTm = (wqkv[:, CH:2 * CH].T @ xT)    # [CH, N]
    v = xT.T @ wqkv[:, 2 * CH:3 * CH]    # [N, CH]
    y = np.empty((CH, N), np.float32)
    for h in range(HH):
        qh = qT[h * DH:(h + 1) * DH, :]
        kh = kTm[h * DH:(h + 1) * DH, :]
        sT = kh.T @ qh                       # [j, i]
        e = np.exp(sT * SCALE)
        den = e.sum(axis=1, keepdims=True)   # over queries i, per key j
        vp = v[:, h * DH:(h + 1) * DH] / den
        y[h * DH:(h + 1) * DH, :] = vp.T @ e
    return (wout.T @ y).astype(np.float16)   # [OUT, N] f16 like device


def _kernel_emulated(x, w_qkv, w_out, b_out):
    in_maps = _shard_inputs(np.asarray(x), np.asarray(w_qkv),
                            np.asarray(w_out), np.asarray(b_out))
    results = [{"outT": _emulate_core(m)} for m in in_maps]
    return _gather_outputs(results, np.asarray(b_out, np.float32))


if __name__ == "__main__":
    # host-logic self-test vs reference math in float64-ish precision
    rng = np.random.default_rng(0)
    x = rng.standard_normal((B, N, F)).astype(np.float32)
    w_qkv = (rng.standard_normal((F, 3 * OUT)) / np.sqrt(F)).astype(np.float32)
    w_out = (rng.standard_normal((OUT, OUT)) / np.sqrt(OUT)).astype(np.float32)
    b_out = np.zeros((OUT,), np.float32)

    def ref(x, w_qkv, w_out, b_out):
        qkv = x @ w_qkv
        q, k, v = np.split(qkv, 3, axis=-1)

        def heads(t):
            return t.reshape(B, N, 16, DH).transpose(0, 2, 1, 3)
        q, k, v = heads(q), heads(k), heads(v)
        s = np.einsum('bhid,bhjd->bhij', q, k) * SCALE
        e = np.exp(s - s.max(axis=2, keepdims=True))
        a = e / e.sum(axis=2, keepdims=True)
        y = np.einsum('bhij,bhjd->bhid', a, v)
        y = y.transpose(0, 2, 1, 3).reshape(B, N, 16 * DH)
        return y @ w_out + b_out

    exp = ref(x, w_qkv, w_out, b_out)
    act = _kernel_emulated(x, w_qkv, w_out, b_out)
    rel = np.linalg.norm(act - exp) / np.linalg.norm(exp)
    print("emulated rel err:", rel)
